# revision 45
# baseline (speedup 1.0000x reference)
"""Trainium2 Bass kernel for nn_Attention_51376398794919.

Dense transformer block: LayerNorm -> QKV -> attention with relative-position
bias -> proj.  Data-parallel over batch across 8 NeuronCores (4 batches/core).

Device-side strategy (per core):
  - LN in natural layout [tok, d]; xn transposed to xnT [d, tok] via PE
    transposes (bf16).
  - qT/kT ([d_head, tok]) and v-natural ([tok, d_v]) computed from xnT;
    q-scale and LN affine folded into the weights on host.
  - Scores computed TRANSPOSED, unpacked: ST[k, q] = kT.T @ qT per k-tile
    (K=64).  exp on the scalar engine PSUM->SBUF (scores are bounded, no
    max-subtraction); the relative-position bias is applied as an in-place
    DVE multiply by host-precomputed exp(bias) (exp(s+b) = exp(s)*exp(b)),
    which keeps the bias addition off the PE entirely.
  - PV: out[q, d|den] = expST.T @ [v | ones]; the ones column yields the
    softmax denominator; normalization fused into the PSUM->SBUF copy
    (DVE tensor_scalar with the reciprocal).
  - Normalized attn [q, d] is staged to a DRAM ring buffer and read back
    TRANSPOSED via the XBAR dma-transpose unit ([dh, tok] tiles), removing
    PE transpose-back work; the epilogue proj consumes those tiles.  The
    LAST head of each batch transposes back on the PE instead, so proj is
    never gated by the DRAM round-trip latency.
  - Two-deep software pipeline over heads: within step i the 16 score
    matmuls of head i+1 are interleaved with the 64 PV matmuls of head i,
    so the scalar-engine exp chain (the PSUM-rotation limiter) drains
    while the PE stays busy; qkv of head i+2 fills the step tail.
    LayerNorm of batch b+1 is spread across the heads of batch b with the
    x DMAs issued two steps before the stats so the in-order DVE queue
    never blocks on a load.
All matmuls run in bf16 with fp32 PSUM accumulation.  K=1 bias matmuls are
emitted only when the corresponding bias is nonzero (host-checked flags).
"""

import sys

import numpy as np

sys.path.insert(0, "/opt/trn_rl_repo")

import concourse.bacc as bacc
import concourse.mybir as mybir
import concourse.tile as tile
from concourse.bass_utils import run_bass_kernel_spmd

# Problem constants
B, N, DIM = 32, 1024, 512
H, KD, D = 8, 64, 256
DH = D * H  # 2048
SCALE = KD ** -0.5
NCORES = 8
BL = B // NCORES  # 4 batches per core

F32 = mybir.dt.float32
BF16 = mybir.dt.bfloat16
AF = mybir.ActivationFunctionType
ALU = mybir.AluOpType

KT = N // 128    # 8 k-tiles
QS = N // 128    # 8 q-slices
DT = DIM // 128  # 4 d-tiles
VW = 257         # v-hat width: 256 v + 1 ones (softmax denominator)


def build_program(use_qk_bias=False, use_v_bias=False, use_pb=False):
    nc = bacc.Bacc("TRN2", target_bir_lowering=False, debug=True)

    x_d = nc.declare_dram_parameter("x", [BL, N, DIM], F32, isOutput=False)
    wqk_d = nc.declare_dram_parameter("wqk", [DIM, H * 128], BF16, isOutput=False)
    wv_d = nc.declare_dram_parameter("wv", [DIM, DH], BF16, isOutput=False)
    bqk_d = nc.declare_dram_parameter("bqk", [1, H * 128], BF16, isOutput=False)
    bv_d = nc.declare_dram_parameter("bv", [1, H * 256], BF16, isOutput=False)
    pw_d = nc.declare_dram_parameter("pw", [DH, DIM], BF16, isOutput=False)
    pb1_d = nc.declare_dram_parameter("pb1", [1, DIM], BF16, isOutput=False)
    eb_d = nc.declare_dram_parameter("eb", [H, N, N], BF16, isOutput=False)
    identb_d = nc.declare_dram_parameter("identb", [128, 128], BF16, isOutput=False)
    ones_d = nc.declare_dram_parameter("ones", [1, 512], BF16, isOutput=False)
    y_d = nc.declare_dram_parameter("y", [BL, N, DIM], F32, isOutput=True)

    with tile.TileContext(nc) as tc:
        with (
            tc.tile_pool(name="consts", bufs=1) as cpool,
            tc.tile_pool(name="xnt", bufs=2) as xpool,
            tc.tile_pool(name="lnx", bufs=4) as xtpool,
            tc.tile_pool(name="lnxn", bufs=4) as xnpool,
            tc.tile_pool(name="stats", bufs=8) as spool,
            tc.tile_pool(name="eb", bufs=2) as ebpool,
            tc.tile_pool(name="qk", bufs=4) as qkpool,
            tc.tile_pool(name="vhat", bufs=3) as vpool,
            tc.tile_pool(name="expst", bufs=16) as epool,
            tc.tile_pool(name="anq", bufs=2) as aqpool,
            tc.tile_pool(name="attnT", bufs=14) as atpool,
            tc.tile_pool(name="yout", bufs=2) as ypool,
            tc.tile_pool(name="stp", bufs=2, space="PSUM") as stpp,
            tc.tile_pool(name="pvp", bufs=2, space="PSUM") as pvpp,
            tc.tile_pool(name="miscp", bufs=2, space="PSUM") as mpp,
            tc.tile_pool(name="dram", bufs=1, space="DRAM") as dpool,
        ):
            # ---- constants (x/identb loads first; pw deferred) ----
            identb = cpool.tile([128, 128], BF16)
            nc.sync.dma_start(identb[:], identb_d[:])
            eps_t = cpool.tile([128, 1], F32)
            nc.vector.memset(eps_t[:], 1e-5)
            zero_t = cpool.tile([128, 1], F32)
            nc.vector.memset(zero_t[:], 0.0)
            if use_qk_bias or use_v_bias or use_pb:
                ones_bf = cpool.tile([1, 512], BF16)
                nc.sync.dma_start(ones_bf[:], ones_d[:])
            if use_qk_bias:
                bqk = cpool.tile([1, H * 128], BF16)
                nc.sync.dma_start(bqk[:], bqk_d[:])
            if use_v_bias:
                bv = cpool.tile([1, H * 256], BF16)
                nc.sync.dma_start(bv[:], bv_d[:])
            if use_pb:
                pb1 = cpool.tile([1, DIM], BF16)
                nc.sync.dma_start(pb1[:], pb1_d[:])
            wqk = cpool.tile([128, DT * H * 128], BF16)  # [d-tile][dpart, f]
            wv = cpool.tile([128, DT * DH], BF16)
            pw = cpool.tile([128, 16 * DIM], BF16)
            slab2 = cpool.tile([128, 2 * N], BF16)  # last head's attnT

            def emit_weight_loads():
                for dt in range(DT):
                    for hh in range(2):
                        nc.sync.dma_start(
                            wqk[:, dt * H * 128 + hh * 512:
                                dt * H * 128 + (hh + 1) * 512],
                            wqk_d[dt * 128:(dt + 1) * 128,
                                  hh * 512:(hh + 1) * 512],
                        )
                for dt in range(DT):
                    for hh in range(2):
                        nc.sync.dma_start(
                            wv[:, dt * DH + hh * (DH // 2):
                               dt * DH + (hh + 1) * (DH // 2)],
                            wv_d[dt * 128:(dt + 1) * 128,
                                 hh * (DH // 2):(hh + 1) * (DH // 2)],
                        )

            def emit_pw_loads():
                for dh in range(16):
                    nc.sync.dma_start(
                        pw[:, dh * DIM:(dh + 1) * DIM],
                        pw_d[dh * 128:(dh + 1) * 128, :],
                    )
            # DRAM ring for the normalized attention (two batches deep)
            attn_dram = dpool.tile([2, N, DH], BF16)

            xnt_tiles = {}

            def get_xnt(b):
                if b not in xnt_tiles:
                    xnt_tiles[b] = xpool.tile(
                        [128, DT * N], BF16, tag="xnt", name="xnt"
                    )
                return xnt_tiles[b]

            def emit_x(b, sl):
                """Issue the x-tile DMA for slice sl of batch b."""
                xt = xtpool.tile([128, DIM], F32, tag="x", name="xt")
                nc.sync.dma_start(xt[:], x_d[b, sl * 128:(sl + 1) * 128, :])
                return xt

            def emit_ln_stats(b, sl, xt):
                """LayerNorm compute (no PE) for a prefetched x tile."""
                st6 = spool.tile([128, 6], F32, tag="st6")
                nc.vector.bn_stats(st6[:], xt[:])
                mv = spool.tile([128, 2], F32, tag="mv")
                nc.vector.bn_aggr(mv[:], st6[:])
                sd = spool.tile([128, 1], F32, tag="sd")
                nc.scalar.activation(sd[:], mv[:, 1:2], AF.Sqrt, bias=eps_t[:])
                rs = spool.tile([128, 1], F32, tag="rs")
                nc.vector.reciprocal(rs[:], sd[:])
                nm = spool.tile([128, 1], F32, tag="nm")
                nc.vector.tensor_scalar(
                    nm[:], mv[:, 0:1], rs[:], -1.0, ALU.mult, ALU.mult
                )
                xn = xnpool.tile([128, DIM], BF16, tag="xn", name="xn")
                nc.vector.tensor_scalar(
                    xn[:], xt[:], rs[:], nm[:], ALU.mult, ALU.add
                )
                return xn

            def emit_ln_tp(b, sl, xn):
                """PE transposes of a prepared LN slice into xnT."""
                xnt = get_xnt(b)
                for dt in range(DT):
                    tp = mpp.tile([128, 128], BF16, tag="m", name="lntp")
                    nc.tensor.transpose(
                        tp[:], xn[:, dt * 128:(dt + 1) * 128], identb[:]
                    )
                    nc.vector.tensor_copy(
                        xnt[:, dt * N + sl * 128: dt * N + (sl + 1) * 128],
                        tp[:],
                    )

            def emit_ln(b, sl):
                emit_ln_tp(b, sl, emit_ln_stats(b, sl, emit_x(b, sl)))

            def emit_score_kt(hctx, est, kt):
                """One k-tile of transposed scores + exp + bias-multiply."""
                qt, ktt, vh, ebh = hctx
                sp = stpp.tile([128, N], F32, tag="st")
                ks = ktt[:, kt * 128:(kt + 1) * 128]
                nc.tensor.matmul(
                    sp[:, 0:512], ks, qt[:, 0:512], start=True, stop=True,
                )
                nc.tensor.matmul(
                    sp[:, 512:1024], ks, qt[:, 512:1024],
                    start=True, stop=True, skip_group_check=True,
                )
                et = epool.tile([128, N], BF16, tag="e")
                nc.scalar.activation(et[:], sp[:], AF.Exp, bias=zero_t[:])
                nc.vector.tensor_tensor(
                    et[:], et[:],
                    ebh[kt // 4][:, (kt % 4) * N:(kt % 4 + 1) * N], ALU.mult,
                )
                est.append(et)

            def emit_qp(b, h, qt, ktt, c):
                """qT/kT chunk c for head h (into partitions 0:64 tiles)."""
                xnt = get_xnt(b)
                qp = mpp.tile([128, 512], F32, tag="m", name="qp")
                for dt in range(DT):
                    nc.tensor.matmul(
                        qp[:],
                        wqk[:, dt * H * 128 + h * 128:
                            dt * H * 128 + (h + 1) * 128],
                        xnt[:, dt * N + c * 512: dt * N + (c + 1) * 512],
                        start=(dt == 0),
                        stop=(not use_qk_bias and dt == DT - 1),
                    )
                if use_qk_bias:
                    nc.tensor.matmul(
                        qp[:],
                        bqk[:, h * 128:(h + 1) * 128],
                        ones_bf[:, 0:512],
                        start=False,
                        stop=True,
                    )
                nc.vector.tensor_copy(qt[:, c * 512:(c + 1) * 512], qp[0:64, :])
                nc.vector.tensor_copy(ktt[:, c * 512:(c + 1) * 512],
                                      qp[64:128, :])

            def emit_v(b, h, vh, sl):
                """v-hat slice sl for head h."""
                xnt = get_xnt(b)
                vp = pvpp.tile([128, VW], F32, tag="pv", name="vp")
                for dt in range(DT):
                    nc.tensor.matmul(
                        vp[:, 0:256],
                        xnt[:, dt * N + sl * 128: dt * N + (sl + 1) * 128],
                        wv[:, dt * DH + h * 256: dt * DH + (h + 1) * 256],
                        start=(dt == 0),
                        stop=(not use_v_bias and dt == DT - 1),
                    )
                if use_v_bias:
                    nc.tensor.matmul(
                        vp[:, 0:256],
                        ones_bf[:, 0:128],
                        bv[:, h * 256:(h + 1) * 256],
                        start=False,
                        stop=True,
                        skip_group_check=True,
                    )
                nc.scalar.copy(vh[:, sl * VW: sl * VW + 256], vp[:, 0:256])

            def emit_eb(nh):
                """Exp-bias DMA prefetch for head nh (two half-head tiles)."""
                halves = []
                for hf in range(2):
                    ebh = ebpool.tile([128, 4 * N], BF16, tag="eb", name="ebh")
                    for g in range(2):  # 2 k-tiles per DMA
                        nc.sync.dma_start(
                            ebh[:, g * 2 * N:(g + 1) * 2 * N]
                            .rearrange("p (kt q) -> p kt q", q=N),
                            eb_d[nh, hf * 512 + g * 256:
                                 hf * 512 + (g + 1) * 256, :]
                            .rearrange("(kt p) q -> p kt q", p=128),
                        )
                    halves.append(ebh)
                return halves

            def emit_qkv_tiles(nh):
                qt2 = qkpool.tile([64, N], BF16, tag="qt")
                ktt2 = qkpool.tile([64, N], BF16, tag="kt")
                vh2 = vpool.tile([128, KT * VW], BF16, tag="vh")
                nc.vector.memset(
                    vh2[:].rearrange("p (s w) -> p s w", w=VW)[:, :, 256:257],
                    1.0,
                )
                return qt2, ktt2, vh2

            def emit_qkv_mm(nb, nh, nctx):
                qt2, ktt2, vh2, _ = nctx
                emit_qp(nb, nh, qt2, ktt2, 0)
                emit_qp(nb, nh, qt2, ktt2, 1)
                for sl in range(QS):
                    emit_v(nb, nh, vh2, sl)

            def emit_v_mm(nb, nh, nctx):
                vh2 = nctx[2]
                for sl in range(QS):
                    emit_v(nb, nh, vh2, sl)

            def emit_pv_sl(hctx, est, anq, sl):
                """One q-slice of PV with fused denominator + normalize."""
                qt, ktt, vh, ebh = hctx
                pv = pvpp.tile([128, VW], F32, tag="pv", name="pv")
                for kt in range(KT):
                    nc.tensor.matmul(
                        pv[:],
                        est[kt][:, sl * 128:(sl + 1) * 128],
                        vh[:, kt * VW:(kt + 1) * VW],
                        start=(kt == 0),
                        stop=(kt == KT - 1),
                    )
                rc = spool.tile([128, 1], F32, tag="rc")
                nc.vector.reciprocal(rc[:], pv[:, 256:257])
                nc.vector.tensor_scalar(
                    anq[:, sl * 256:(sl + 1) * 256],
                    pv[:, 0:256], rc[:], None, ALU.mult,
                )

            def emit_head_main(sctx, est_next, hctx, est, anq):
                """Interleave scores/exp of head i+1 with PV of head i:
                the PV matmuls keep the PE busy while the ACT exp chain
                drains the score PSUM tiles."""
                if sctx is not None:
                    emit_score_kt(sctx, est_next, 0)
                    emit_score_kt(sctx, est_next, 1)
                    for kt in range(2, KT):
                        emit_pv_sl(hctx, est, anq, kt - 2)
                        emit_score_kt(sctx, est_next, kt)
                    emit_pv_sl(hctx, est, anq, 6)
                    emit_pv_sl(hctx, est, anq, 7)
                else:
                    for sl in range(QS):
                        emit_pv_sl(hctx, est, anq, sl)

            def emit_attn_write(b, h, anq):
                rb = b % 2
                nc.sync.dma_start(
                    attn_dram[rb, :, h * 256:(h + 1) * 256]
                    .rearrange("(s p) d -> p s d", p=128),
                    anq[:].rearrange("p (s d) -> p s d", d=256),
                )

            def emit_attn_reads(b, h, attns):
                rb = b % 2
                for dc in range(2):
                    at = atpool.tile([128, N], BF16, tag="at")
                    nc.sync.dma_start_transpose(
                        at[:],
                        attn_dram[rb, :, (h * 2 + dc) * 128:
                                  (h * 2 + dc + 1) * 128],
                    )
                    attns.append((at, 0))

            def emit_attn_tb_last(anq, attns):
                """PE transpose-back for the last head (avoids the DRAM
                round-trip latency right before proj)."""
                for sl in range(QS):
                    for dc in range(2):
                        tp = mpp.tile([128, 128], BF16, tag="m", name="tb")
                        nc.tensor.transpose(
                            tp[:],
                            anq[:, sl * 256 + dc * 128:
                                sl * 256 + (dc + 1) * 128],
                            identb[:],
                        )
                        nc.vector.tensor_copy(
                            slab2[:, dc * N + sl * 128:
                                  dc * N + (sl + 1) * 128],
                            tp[:],
                        )
                attns.append((slab2, 0))
                attns.append((slab2, N))

            def emit_proj(b, attns):
                for sl in range(QS):
                    pp = mpp.tile([128, 512], F32, tag="m", name="pp")
                    for dh in range(16):
                        t, base = attns[dh]
                        nc.tensor.matmul(
                            pp[:],
                            t[:, base + sl * 128: base + (sl + 1) * 128],
                            pw[:, dh * DIM:(dh + 1) * DIM],
                            start=(dh == 0),
                            stop=(not use_pb and dh == 15),
                        )
                    if use_pb:
                        nc.tensor.matmul(
                            pp[:], ones_bf[:, 0:128], pb1[:], start=False,
                            stop=True, skip_group_check=True,
                        )
                    yt = ypool.tile([128, DIM], F32, tag="y")
                    nc.scalar.copy(yt[:], pp[:])
                    nc.sync.dma_start(y_d[b, sl * 128:(sl + 1) * 128, :], yt[:])

            # ---- main pipeline ----
            # Global head index i = b*H + h.  Software pipeline depth 2:
            # scores/exp for head i+1 (interleaved with qkv of head i+2)
            # are emitted before PV(i), so the ACT exp chain of i+1
            # executes during PV(i)/v(i+2) and never gates PV(i+1).
            NH = BL * H

            def bh(i):
                return i // H, i % H

            for sl in range(QS):
                emit_ln(0, sl)
            emit_weight_loads()
            hctxs = {0: emit_qkv_tiles(0) + (emit_eb(0),)}
            emit_qkv_mm(0, 0, hctxs[0])
            ests = {0: []}
            for kt in range(KT):
                emit_score_kt(hctxs[0], ests[0], kt)
            hctxs[1] = emit_qkv_tiles(1) + (emit_eb(1),)
            emit_qkv_mm(*bh(1), hctxs[1])
            emit_pw_loads()
            # Next-batch LayerNorm staging: x-DMA two steps before the PE
            # transposes, stats in between, so the in-order DVE queue never
            # blocks on an x load.
            X_SCHED = {0: [0, 1], 1: [2, 3], 2: [4], 3: [5], 4: [6], 5: [7]}
            ST_SCHED = {1: [0, 1], 2: [2, 3], 3: [4], 4: [5], 5: [6, 7]}
            attns = []
            pending_tp = []
            xts = {}
            for i in range(NH):
                b, h = bh(i)
                # PE transposes of last step's prepared LN slices
                for sl, xn in pending_tp:
                    emit_ln_tp(b + 1, sl, xn)
                pending_tp = []
                if b + 1 < BL:
                    for sl in X_SCHED.get(h, []):
                        xts[sl] = emit_x(b + 1, sl)
                if h > 0:
                    emit_attn_reads(b, h - 1, attns)
                # exp-bias prefetch for head i+2
                if i + 2 < NH:
                    hctxs[i + 2] = emit_qkv_tiles(bh(i + 2)[1]) + (
                        emit_eb(bh(i + 2)[1]),
                    )
                # interleaved scores(i+1) + PV(i)
                anq = aqpool.tile([128, QS * 256], BF16, tag="anq")
                sctx = hctxs[i + 1] if i + 1 < NH else None
                est_next = []
                emit_head_main(sctx, est_next, hctxs.pop(i), ests.pop(i), anq)
                if i + 1 < NH:
                    ests[i + 1] = est_next
                if h == H - 1:
                    emit_attn_tb_last(anq, attns)
                else:
                    emit_attn_write(b, h, anq)
                # qkv matmuls for head i+2 (tail of the step)
                if i + 2 < NH:
                    emit_qkv_mm(*bh(i + 2), hctxs[i + 2])
                if h == H - 1:
                    emit_proj(b, attns)
                    attns = []
                    xnt_tiles.pop(b, None)
                # LN stats last (slack: needed a step later)
                if b + 1 < BL:
                    for sl in ST_SCHED.get(h, []):
                        pending_tp.append(
                            (sl, emit_ln_stats(b + 1, sl, xts.pop(sl)))
                        )

    nc.compile()
    return nc


_CACHE = {}


def _prep_host(gamma, beta, qkv_w, qkv_b, proj_w, proj_b, biases, bias_idxs):
    import ml_dtypes

    qkv_w = np.asarray(qkv_w, np.float32)
    qkv_b = np.asarray(qkv_b, np.float32)
    gamma = np.asarray(gamma, np.float32)
    beta = np.asarray(beta, np.float32)
    w = qkv_w * gamma[:, None]          # fold LN gamma
    bfold = qkv_b + beta @ qkv_w        # fold LN beta
    w3 = w.reshape(DIM, H, 384)
    b3 = bfold.reshape(H, 384)
    # q/k columns, q scaled by SCALE
    wqk = np.concatenate([w3[:, :, :64] * SCALE, w3[:, :, 64:128]], axis=2)
    wqk = wqk.reshape(DIM, H * 128)
    bqk = np.concatenate([b3[:, :64] * SCALE, b3[:, 64:128]], axis=1)
    bqk = bqk.reshape(1, H * 128)
    wv = w3[:, :, 128:].reshape(DIM, DH)
    bv = b3[:, 128:].reshape(1, H * 256)
    bias_full = np.asarray(biases, np.float32)[:, np.asarray(bias_idxs)]
    # device reads bias tiles as [k, q]; transpose (a no-op for the
    # symmetric relative-position bias, but correct in general)
    eb = np.exp(bias_full.transpose(0, 2, 1))
    return {
        "wqk": wqk.astype(ml_dtypes.bfloat16),
        "wv": wv.astype(ml_dtypes.bfloat16),
        "bqk": bqk.astype(ml_dtypes.bfloat16),
        "bv": bv.astype(ml_dtypes.bfloat16),
        "pw": np.ascontiguousarray(np.asarray(proj_w, np.float32)).astype(ml_dtypes.bfloat16),
        "pb1": np.asarray(proj_b, np.float32).reshape(1, DIM).astype(ml_dtypes.bfloat16),
        "eb": np.ascontiguousarray(eb).astype(ml_dtypes.bfloat16),
        "identb": np.eye(128, dtype=np.float32).astype(ml_dtypes.bfloat16),
        "ones": np.ones((1, 512), ml_dtypes.bfloat16),
    }


def kernel(x, gamma, beta, qkv_w, qkv_b, proj_w, proj_b, biases, bias_idxs,
           _trace=False, _tmpdir=None):
    x = np.asarray(x, np.float32)
    shared = _prep_host(gamma, beta, qkv_w, qkv_b, proj_w, proj_b, biases,
                        bias_idxs)
    flags = (
        bool(np.any(np.asarray(shared["bqk"], np.float32))),
        bool(np.any(np.asarray(shared["bv"], np.float32))),
        bool(np.any(np.asarray(shared["pb1"], np.float32))),
    )
    if _CACHE.get("flags") != flags:
        _CACHE["nc"] = build_program(*flags)
        _CACHE["flags"] = flags
    nc = _CACHE["nc"]
    in_maps = []
    for c in range(NCORES):
        m = dict(shared)
        m["x"] = np.ascontiguousarray(x[c * BL:(c + 1) * BL])
        in_maps.append(m)
    res = run_bass_kernel_spmd(
        nc, in_maps, list(range(NCORES)), trace=_trace, tmpdir=_tmpdir,
    )
    _CACHE["last"] = res
    out = np.concatenate([res.results[c]["y"] for c in range(NCORES)], axis=0)
    return out.astype(np.float32)


# revision 47
# speedup vs baseline: 1.0002x; 1.0002x over previous
"""Trainium2 Bass kernel for nn_Attention_51376398794919.

Dense transformer block: LayerNorm -> QKV -> attention with relative-position
bias -> proj.  Data-parallel over batch across 8 NeuronCores (4 batches/core).

Device-side strategy (per core):
  - LN in natural layout [tok, d]; xn transposed to xnT [d, tok] via PE
    transposes (bf16).
  - qT/kT ([d_head, tok]) and v-natural ([tok, d_v]) computed from xnT;
    q-scale and LN affine folded into the weights on host.
  - Scores computed TRANSPOSED, unpacked: ST[k, q] = kT.T @ qT per k-tile
    (K=64).  exp on the scalar engine PSUM->SBUF (scores are bounded, no
    max-subtraction); the relative-position bias is applied as an in-place
    DVE multiply by host-precomputed exp(bias) (exp(s+b) = exp(s)*exp(b)),
    which keeps the bias addition off the PE entirely.
  - PV: out[q, d|den] = expST.T @ [v | ones]; the ones column yields the
    softmax denominator; normalization fused into the PSUM->SBUF copy
    (DVE tensor_scalar with the reciprocal).
  - Normalized attn [q, d] is staged to a DRAM ring buffer and read back
    TRANSPOSED via the XBAR dma-transpose unit ([dh, tok] tiles), removing
    PE transpose-back work; the epilogue proj consumes those tiles.  The
    LAST head of each batch transposes back on the PE instead, so proj is
    never gated by the DRAM round-trip latency.
  - Two-deep software pipeline over heads: within step i the 16 score
    matmuls of head i+1 are interleaved with the 64 PV matmuls of head i,
    so the scalar-engine exp chain (the PSUM-rotation limiter) drains
    while the PE stays busy; qkv of head i+2 fills the step tail.
    LayerNorm of batch b+1 is spread across the heads of batch b with the
    x DMAs issued two steps before the stats so the in-order DVE queue
    never blocks on a load.
All matmuls run in bf16 with fp32 PSUM accumulation.  K=1 bias matmuls are
emitted only when the corresponding bias is nonzero (host-checked flags).
"""

import sys

import numpy as np

sys.path.insert(0, "/opt/trn_rl_repo")

import concourse.bacc as bacc
import concourse.mybir as mybir
import concourse.tile as tile
from concourse.bass_utils import run_bass_kernel_spmd

# Problem constants
B, N, DIM = 32, 1024, 512
H, KD, D = 8, 64, 256
DH = D * H  # 2048
SCALE = KD ** -0.5
NCORES = 8
BL = B // NCORES  # 4 batches per core

F32 = mybir.dt.float32
BF16 = mybir.dt.bfloat16
AF = mybir.ActivationFunctionType
ALU = mybir.AluOpType

KT = N // 128    # 8 k-tiles
QS = N // 128    # 8 q-slices
DT = DIM // 128  # 4 d-tiles
VW = 257         # v-hat width: 256 v + 1 ones (softmax denominator)


def build_program(use_qk_bias=False, use_v_bias=False, use_pb=False):
    nc = bacc.Bacc("TRN2", target_bir_lowering=False, debug=True)

    x_d = nc.declare_dram_parameter("x", [BL, N, DIM], F32, isOutput=False)
    wqk_d = nc.declare_dram_parameter("wqk", [DIM, H * 128], BF16, isOutput=False)
    wv_d = nc.declare_dram_parameter("wv", [DIM, DH], BF16, isOutput=False)
    bqk_d = nc.declare_dram_parameter("bqk", [1, H * 128], BF16, isOutput=False)
    bv_d = nc.declare_dram_parameter("bv", [1, H * 256], BF16, isOutput=False)
    pw_d = nc.declare_dram_parameter("pw", [DH, DIM], BF16, isOutput=False)
    pb1_d = nc.declare_dram_parameter("pb1", [1, DIM], BF16, isOutput=False)
    eb_d = nc.declare_dram_parameter("eb", [H, N, N], BF16, isOutput=False)
    identb_d = nc.declare_dram_parameter("identb", [128, 128], BF16, isOutput=False)
    ones_d = nc.declare_dram_parameter("ones", [1, 512], BF16, isOutput=False)
    y_d = nc.declare_dram_parameter("y", [BL, N, DIM], F32, isOutput=True)

    with tile.TileContext(nc) as tc:
        with (
            tc.tile_pool(name="consts", bufs=1) as cpool,
            tc.tile_pool(name="xnt", bufs=2) as xpool,
            tc.tile_pool(name="lnx", bufs=4) as xtpool,
            tc.tile_pool(name="lnxn", bufs=5) as xnpool,
            tc.tile_pool(name="stats", bufs=16) as spool,
            tc.tile_pool(name="eb", bufs=2) as ebpool,
            tc.tile_pool(name="qk", bufs=4) as qkpool,
            tc.tile_pool(name="vhat", bufs=3) as vpool,
            tc.tile_pool(name="expst", bufs=16) as epool,
            tc.tile_pool(name="anq", bufs=2) as aqpool,
            tc.tile_pool(name="attnT", bufs=14) as atpool,
            tc.tile_pool(name="yout", bufs=2) as ypool,
            tc.tile_pool(name="stp", bufs=2, space="PSUM") as stpp,
            tc.tile_pool(name="pvp", bufs=2, space="PSUM") as pvpp,
            tc.tile_pool(name="miscp", bufs=2, space="PSUM") as mpp,
            tc.tile_pool(name="dram", bufs=1, space="DRAM") as dpool,
        ):
            # ---- constants (x/identb loads first; pw deferred) ----
            identb = cpool.tile([128, 128], BF16)
            nc.sync.dma_start(identb[:], identb_d[:])
            eps_t = cpool.tile([128, 1], F32)
            nc.vector.memset(eps_t[:], 1e-5)
            zero_t = cpool.tile([128, 1], F32)
            nc.vector.memset(zero_t[:], 0.0)
            if use_qk_bias or use_v_bias or use_pb:
                ones_bf = cpool.tile([1, 512], BF16)
                nc.sync.dma_start(ones_bf[:], ones_d[:])
            if use_qk_bias:
                bqk = cpool.tile([1, H * 128], BF16)
                nc.sync.dma_start(bqk[:], bqk_d[:])
            if use_v_bias:
                bv = cpool.tile([1, H * 256], BF16)
                nc.sync.dma_start(bv[:], bv_d[:])
            if use_pb:
                pb1 = cpool.tile([1, DIM], BF16)
                nc.sync.dma_start(pb1[:], pb1_d[:])
            wqk = cpool.tile([128, DT * H * 128], BF16)  # [d-tile][dpart, f]
            wv = cpool.tile([128, DT * DH], BF16)
            pw = cpool.tile([128, 16 * DIM], BF16)
            slab2 = cpool.tile([128, 2 * N], BF16)  # last head's attnT

            def emit_weight_loads():
                for dt in range(DT):
                    for hh in range(2):
                        nc.sync.dma_start(
                            wqk[:, dt * H * 128 + hh * 512:
                                dt * H * 128 + (hh + 1) * 512],
                            wqk_d[dt * 128:(dt + 1) * 128,
                                  hh * 512:(hh + 1) * 512],
                        )
                for dt in range(DT):
                    for hh in range(2):
                        nc.sync.dma_start(
                            wv[:, dt * DH + hh * (DH // 2):
                               dt * DH + (hh + 1) * (DH // 2)],
                            wv_d[dt * 128:(dt + 1) * 128,
                                 hh * (DH // 2):(hh + 1) * (DH // 2)],
                        )

            def emit_pw_loads():
                for dh in range(16):
                    nc.sync.dma_start(
                        pw[:, dh * DIM:(dh + 1) * DIM],
                        pw_d[dh * 128:(dh + 1) * 128, :],
                    )
            # DRAM ring for the normalized attention (two batches deep)
            attn_dram = dpool.tile([2, N, DH], BF16)

            xnt_tiles = {}

            def get_xnt(b):
                if b not in xnt_tiles:
                    xnt_tiles[b] = xpool.tile(
                        [128, DT * N], BF16, tag="xnt", name="xnt"
                    )
                return xnt_tiles[b]

            def emit_x(b, sl):
                """Issue the x-tile DMA for slice sl of batch b."""
                xt = xtpool.tile([128, DIM], F32, tag="x", name="xt")
                nc.sync.dma_start(xt[:], x_d[b, sl * 128:(sl + 1) * 128, :])
                return xt

            def emit_ln_stats(b, sl, xt):
                """LayerNorm compute (no PE) for a prefetched x tile."""
                st6 = spool.tile([128, 6], F32, tag="st6")
                nc.vector.bn_stats(st6[:], xt[:])
                mv = spool.tile([128, 2], F32, tag="mv")
                nc.vector.bn_aggr(mv[:], st6[:])
                sd = spool.tile([128, 1], F32, tag="sd")
                nc.scalar.activation(sd[:], mv[:, 1:2], AF.Sqrt, bias=eps_t[:])
                rs = spool.tile([128, 1], F32, tag="rs")
                nc.vector.reciprocal(rs[:], sd[:])
                nm = spool.tile([128, 1], F32, tag="nm")
                nc.vector.tensor_scalar(
                    nm[:], mv[:, 0:1], rs[:], -1.0, ALU.mult, ALU.mult
                )
                xn = xnpool.tile([128, DIM], BF16, tag="xn", name="xn")
                nc.vector.tensor_scalar(
                    xn[:], xt[:], rs[:], nm[:], ALU.mult, ALU.add
                )
                return xn

            def emit_ln_tp(b, sl, xn):
                """PE transposes of a prepared LN slice into xnT."""
                xnt = get_xnt(b)
                for dt in range(DT):
                    tp = mpp.tile([128, 128], BF16, tag="m", name="lntp")
                    nc.tensor.transpose(
                        tp[:], xn[:, dt * 128:(dt + 1) * 128], identb[:]
                    )
                    nc.vector.tensor_copy(
                        xnt[:, dt * N + sl * 128: dt * N + (sl + 1) * 128],
                        tp[:],
                    )

            def emit_ln(b, sl):
                emit_ln_tp(b, sl, emit_ln_stats(b, sl, emit_x(b, sl)))

            def emit_score_kt(hctx, est, kt):
                """One k-tile of transposed scores + exp + bias-multiply."""
                qt, ktt, vh, ebh = hctx
                sp = stpp.tile([128, N], F32, tag="st")
                ks = ktt[:, kt * 128:(kt + 1) * 128]
                nc.tensor.matmul(
                    sp[:, 0:512], ks, qt[:, 0:512], start=True, stop=True,
                )
                nc.tensor.matmul(
                    sp[:, 512:1024], ks, qt[:, 512:1024],
                    start=True, stop=True, skip_group_check=True,
                )
                et = epool.tile([128, N], BF16, tag="e")
                nc.scalar.activation(et[:], sp[:], AF.Exp, bias=zero_t[:])
                nc.vector.tensor_tensor(
                    et[:], et[:],
                    ebh[kt // 4][:, (kt % 4) * N:(kt % 4 + 1) * N], ALU.mult,
                )
                est.append(et)

            def emit_qp(b, h, qt, ktt, c):
                """qT/kT chunk c for head h (into partitions 0:64 tiles)."""
                xnt = get_xnt(b)
                qp = mpp.tile([128, 512], F32, tag="m", name="qp")
                for dt in range(DT):
                    nc.tensor.matmul(
                        qp[:],
                        wqk[:, dt * H * 128 + h * 128:
                            dt * H * 128 + (h + 1) * 128],
                        xnt[:, dt * N + c * 512: dt * N + (c + 1) * 512],
                        start=(dt == 0),
                        stop=(not use_qk_bias and dt == DT - 1),
                    )
                if use_qk_bias:
                    nc.tensor.matmul(
                        qp[:],
                        bqk[:, h * 128:(h + 1) * 128],
                        ones_bf[:, 0:512],
                        start=False,
                        stop=True,
                    )
                nc.vector.tensor_copy(qt[:, c * 512:(c + 1) * 512], qp[0:64, :])
                nc.vector.tensor_copy(ktt[:, c * 512:(c + 1) * 512],
                                      qp[64:128, :])

            def emit_v(b, h, vh, sl):
                """v-hat slice sl for head h."""
                xnt = get_xnt(b)
                vp = pvpp.tile([128, VW], F32, tag="pv", name="vp")
                for dt in range(DT):
                    nc.tensor.matmul(
                        vp[:, 0:256],
                        xnt[:, dt * N + sl * 128: dt * N + (sl + 1) * 128],
                        wv[:, dt * DH + h * 256: dt * DH + (h + 1) * 256],
                        start=(dt == 0),
                        stop=(not use_v_bias and dt == DT - 1),
                    )
                if use_v_bias:
                    nc.tensor.matmul(
                        vp[:, 0:256],
                        ones_bf[:, 0:128],
                        bv[:, h * 256:(h + 1) * 256],
                        start=False,
                        stop=True,
                        skip_group_check=True,
                    )
                nc.scalar.copy(vh[:, sl * VW: sl * VW + 256], vp[:, 0:256])

            def emit_eb(nh):
                """Exp-bias DMA prefetch for head nh (two half-head tiles)."""
                halves = []
                for hf in range(2):
                    ebh = ebpool.tile([128, 4 * N], BF16, tag="eb", name="ebh")
                    for g in range(2):  # 2 k-tiles per DMA
                        nc.sync.dma_start(
                            ebh[:, g * 2 * N:(g + 1) * 2 * N]
                            .rearrange("p (kt q) -> p kt q", q=N),
                            eb_d[nh, hf * 512 + g * 256:
                                 hf * 512 + (g + 1) * 256, :]
                            .rearrange("(kt p) q -> p kt q", p=128),
                        )
                    halves.append(ebh)
                return halves

            def emit_qkv_tiles(nh):
                qt2 = qkpool.tile([64, N], BF16, tag="qt")
                ktt2 = qkpool.tile([64, N], BF16, tag="kt")
                vh2 = vpool.tile([128, KT * VW], BF16, tag="vh")
                nc.vector.memset(
                    vh2[:].rearrange("p (s w) -> p s w", w=VW)[:, :, 256:257],
                    1.0,
                )
                return qt2, ktt2, vh2

            def emit_qkv_mm(nb, nh, nctx):
                qt2, ktt2, vh2, _ = nctx
                emit_qp(nb, nh, qt2, ktt2, 0)
                emit_qp(nb, nh, qt2, ktt2, 1)
                for sl in range(QS):
                    emit_v(nb, nh, vh2, sl)

            def emit_v_mm(nb, nh, nctx):
                vh2 = nctx[2]
                for sl in range(QS):
                    emit_v(nb, nh, vh2, sl)

            def emit_pv_sl(hctx, est, anq, sl):
                """One q-slice of PV with fused denominator + normalize."""
                qt, ktt, vh, ebh = hctx
                pv = pvpp.tile([128, VW], F32, tag="pv", name="pv")
                for kt in range(KT):
                    nc.tensor.matmul(
                        pv[:],
                        est[kt][:, sl * 128:(sl + 1) * 128],
                        vh[:, kt * VW:(kt + 1) * VW],
                        start=(kt == 0),
                        stop=(kt == KT - 1),
                    )
                rc = spool.tile([128, 1], F32, tag="rc")
                nc.vector.reciprocal(rc[:], pv[:, 256:257])
                nc.vector.tensor_scalar(
                    anq[:, sl * 256:(sl + 1) * 256],
                    pv[:, 0:256], rc[:], None, ALU.mult,
                )

            def emit_head_main(sctx, est_next, hctx, est, anq):
                """Interleave scores/exp of head i+1 with PV of head i:
                the PV matmuls keep the PE busy while the ACT exp chain
                drains the score PSUM tiles."""
                if sctx is not None:
                    emit_score_kt(sctx, est_next, 0)
                    emit_score_kt(sctx, est_next, 1)
                    for kt in range(2, KT):
                        emit_pv_sl(hctx, est, anq, kt - 2)
                        emit_score_kt(sctx, est_next, kt)
                    emit_pv_sl(hctx, est, anq, 6)
                    emit_pv_sl(hctx, est, anq, 7)
                else:
                    for sl in range(QS):
                        emit_pv_sl(hctx, est, anq, sl)

            def emit_attn_write(b, h, anq):
                rb = b % 2
                nc.sync.dma_start(
                    attn_dram[rb, :, h * 256:(h + 1) * 256]
                    .rearrange("(s p) d -> p s d", p=128),
                    anq[:].rearrange("p (s d) -> p s d", d=256),
                )

            def emit_attn_reads(b, h, attns):
                rb = b % 2
                for dc in range(2):
                    at = atpool.tile([128, N], BF16, tag="at")
                    nc.sync.dma_start_transpose(
                        at[:],
                        attn_dram[rb, :, (h * 2 + dc) * 128:
                                  (h * 2 + dc + 1) * 128],
                    )
                    attns.append((at, 0))

            def emit_attn_tb_last(anq, attns):
                """PE transpose-back for the last head (avoids the DRAM
                round-trip latency right before proj)."""
                for sl in range(QS):
                    for dc in range(2):
                        tp = mpp.tile([128, 128], BF16, tag="m", name="tb")
                        nc.tensor.transpose(
                            tp[:],
                            anq[:, sl * 256 + dc * 128:
                                sl * 256 + (dc + 1) * 128],
                            identb[:],
                        )
                        nc.vector.tensor_copy(
                            slab2[:, dc * N + sl * 128:
                                  dc * N + (sl + 1) * 128],
                            tp[:],
                        )
                attns.append((slab2, 0))
                attns.append((slab2, N))

            def emit_proj(b, attns):
                for sl in range(QS):
                    pp = mpp.tile([128, 512], F32, tag="m", name="pp")
                    for dh in range(16):
                        t, base = attns[dh]
                        nc.tensor.matmul(
                            pp[:],
                            t[:, base + sl * 128: base + (sl + 1) * 128],
                            pw[:, dh * DIM:(dh + 1) * DIM],
                            start=(dh == 0),
                            stop=(not use_pb and dh == 15),
                        )
                    if use_pb:
                        nc.tensor.matmul(
                            pp[:], ones_bf[:, 0:128], pb1[:], start=False,
                            stop=True, skip_group_check=True,
                        )
                    yt = ypool.tile([128, DIM], F32, tag="y")
                    nc.scalar.copy(yt[:], pp[:])
                    nc.sync.dma_start(y_d[b, sl * 128:(sl + 1) * 128, :], yt[:])

            # ---- main pipeline ----
            # Global head index i = b*H + h.  Software pipeline depth 2:
            # scores/exp for head i+1 (interleaved with qkv of head i+2)
            # are emitted before PV(i), so the ACT exp chain of i+1
            # executes during PV(i)/v(i+2) and never gates PV(i+1).
            NH = BL * H

            def bh(i):
                return i // H, i % H

            for sl in range(QS):
                emit_ln(0, sl)
            emit_weight_loads()
            hctxs = {0: emit_qkv_tiles(0) + (emit_eb(0),)}
            emit_qkv_mm(0, 0, hctxs[0])
            ests = {0: []}
            for kt in range(KT):
                emit_score_kt(hctxs[0], ests[0], kt)
            hctxs[1] = emit_qkv_tiles(1) + (emit_eb(1),)
            emit_qkv_mm(*bh(1), hctxs[1])
            emit_pw_loads()
            # Next-batch LayerNorm staging: x-DMA two steps before the PE
            # transposes, stats in between, so the in-order DVE queue never
            # blocks on an x load.
            X_SCHED = {0: [0, 1], 1: [2, 3], 2: [4], 3: [5], 4: [6], 5: [7]}
            ST_SCHED = {1: [0, 1], 2: [2, 3], 3: [4], 4: [5], 5: [6, 7]}
            attns = []
            pending_tp = []
            xts = {}
            for i in range(NH):
                b, h = bh(i)
                # PE transposes of last step's prepared LN slices
                for sl, xn in pending_tp:
                    emit_ln_tp(b + 1, sl, xn)
                pending_tp = []
                if b + 1 < BL:
                    for sl in X_SCHED.get(h, []):
                        xts[sl] = emit_x(b + 1, sl)
                if h > 0:
                    emit_attn_reads(b, h - 1, attns)
                # exp-bias prefetch for head i+2
                if i + 2 < NH:
                    hctxs[i + 2] = emit_qkv_tiles(bh(i + 2)[1]) + (
                        emit_eb(bh(i + 2)[1]),
                    )
                # interleaved scores(i+1) + PV(i)
                anq = aqpool.tile([128, QS * 256], BF16, tag="anq")
                sctx = hctxs[i + 1] if i + 1 < NH else None
                est_next = []
                emit_head_main(sctx, est_next, hctxs.pop(i), ests.pop(i), anq)
                if i + 1 < NH:
                    ests[i + 1] = est_next
                if h == H - 1:
                    emit_attn_tb_last(anq, attns)
                else:
                    emit_attn_write(b, h, anq)
                # qkv matmuls for head i+2 (tail of the step)
                if i + 2 < NH:
                    emit_qkv_mm(*bh(i + 2), hctxs[i + 2])
                if h == H - 1:
                    emit_proj(b, attns)
                    attns = []
                    xnt_tiles.pop(b, None)
                # LN stats last (slack: needed a step later)
                if b + 1 < BL:
                    for sl in ST_SCHED.get(h, []):
                        pending_tp.append(
                            (sl, emit_ln_stats(b + 1, sl, xts.pop(sl)))
                        )

    nc.compile()
    return nc


_CACHE = {}


def _prep_host(gamma, beta, qkv_w, qkv_b, proj_w, proj_b, biases, bias_idxs):
    import ml_dtypes

    qkv_w = np.asarray(qkv_w, np.float32)
    qkv_b = np.asarray(qkv_b, np.float32)
    gamma = np.asarray(gamma, np.float32)
    beta = np.asarray(beta, np.float32)
    w = qkv_w * gamma[:, None]          # fold LN gamma
    bfold = qkv_b + beta @ qkv_w        # fold LN beta
    w3 = w.reshape(DIM, H, 384)
    b3 = bfold.reshape(H, 384)
    # q/k columns, q scaled by SCALE
    wqk = np.concatenate([w3[:, :, :64] * SCALE, w3[:, :, 64:128]], axis=2)
    wqk = wqk.reshape(DIM, H * 128)
    bqk = np.concatenate([b3[:, :64] * SCALE, b3[:, 64:128]], axis=1)
    bqk = bqk.reshape(1, H * 128)
    wv = w3[:, :, 128:].reshape(DIM, DH)
    bv = b3[:, 128:].reshape(1, H * 256)
    bias_full = np.asarray(biases, np.float32)[:, np.asarray(bias_idxs)]
    # device reads bias tiles as [k, q]; transpose (a no-op for the
    # symmetric relative-position bias, but correct in general)
    eb = np.exp(bias_full.transpose(0, 2, 1))
    return {
        "wqk": wqk.astype(ml_dtypes.bfloat16),
        "wv": wv.astype(ml_dtypes.bfloat16),
        "bqk": bqk.astype(ml_dtypes.bfloat16),
        "bv": bv.astype(ml_dtypes.bfloat16),
        "pw": np.ascontiguousarray(np.asarray(proj_w, np.float32)).astype(ml_dtypes.bfloat16),
        "pb1": np.asarray(proj_b, np.float32).reshape(1, DIM).astype(ml_dtypes.bfloat16),
        "eb": np.ascontiguousarray(eb).astype(ml_dtypes.bfloat16),
        "identb": np.eye(128, dtype=np.float32).astype(ml_dtypes.bfloat16),
        "ones": np.ones((1, 512), ml_dtypes.bfloat16),
    }


def kernel(x, gamma, beta, qkv_w, qkv_b, proj_w, proj_b, biases, bias_idxs,
           _trace=False, _tmpdir=None):
    x = np.asarray(x, np.float32)
    shared = _prep_host(gamma, beta, qkv_w, qkv_b, proj_w, proj_b, biases,
                        bias_idxs)
    flags = (
        bool(np.any(np.asarray(shared["bqk"], np.float32))),
        bool(np.any(np.asarray(shared["bv"], np.float32))),
        bool(np.any(np.asarray(shared["pb1"], np.float32))),
    )
    if _CACHE.get("flags") != flags:
        _CACHE["nc"] = build_program(*flags)
        _CACHE["flags"] = flags
    nc = _CACHE["nc"]
    in_maps = []
    for c in range(NCORES):
        m = dict(shared)
        m["x"] = np.ascontiguousarray(x[c * BL:(c + 1) * BL])
        in_maps.append(m)
    res = run_bass_kernel_spmd(
        nc, in_maps, list(range(NCORES)), trace=_trace, tmpdir=_tmpdir,
    )
    _CACHE["last"] = res
    out = np.concatenate([res.results[c]["y"] for c in range(NCORES)], axis=0)
    return out.astype(np.float32)


# revision 57
# speedup vs baseline: 1.0043x; 1.0040x over previous
"""Trainium2 Bass kernel for nn_Attention_51376398794919.

Dense transformer block: LayerNorm -> QKV -> attention with relative-position
bias -> proj.  Data-parallel over batch across 8 NeuronCores (4 batches/core).

Device-side strategy (per core):
  - LN in natural layout [tok, d]; xn transposed to xnT [d, tok] via PE
    transposes (bf16).
  - qT/kT ([d_head, tok]) and v-natural ([tok, d_v]) computed from xnT;
    q-scale and LN affine folded into the weights on host.
  - Scores computed TRANSPOSED, unpacked: ST[k, q] = kT.T @ qT per k-tile
    (K=64).  exp on the scalar engine PSUM->SBUF (scores are bounded, no
    max-subtraction); the relative-position bias is applied as an in-place
    DVE multiply by host-precomputed exp(bias) (exp(s+b) = exp(s)*exp(b)),
    which keeps the bias addition off the PE entirely.
  - PV: out[q, d|den] = expST.T @ [v | ones]; the ones column yields the
    softmax denominator; normalization fused into the PSUM->SBUF copy
    (DVE tensor_scalar with the reciprocal).
  - Normalized attn [q, d] is staged to a DRAM ring buffer and read back
    TRANSPOSED via the XBAR dma-transpose unit ([dh, tok] tiles), removing
    PE transpose-back work; the epilogue proj consumes those tiles.  The
    LAST head of each batch transposes back on the PE instead, so proj is
    never gated by the DRAM round-trip latency.
  - Two-deep software pipeline over heads: within step i the 16 score
    matmuls of head i+1 are interleaved with the 64 PV matmuls of head i,
    so the scalar-engine exp chain (the PSUM-rotation limiter) drains
    while the PE stays busy; qkv of head i+2 fills the step tail.
    LayerNorm of batch b+1 is spread across the heads of batch b with the
    x DMAs issued two steps before the stats so the in-order DVE queue
    never blocks on a load.
All matmuls run in bf16 with fp32 PSUM accumulation.  K=1 bias matmuls are
emitted only when the corresponding bias is nonzero (host-checked flags).
"""

import sys

import numpy as np

sys.path.insert(0, "/opt/trn_rl_repo")

import concourse.bacc as bacc
import concourse.mybir as mybir
import concourse.tile as tile
from concourse.bass_utils import run_bass_kernel_spmd

# Problem constants
B, N, DIM = 32, 1024, 512
H, KD, D = 8, 64, 256
DH = D * H  # 2048
SCALE = KD ** -0.5
NCORES = 8
BL = B // NCORES  # 4 batches per core

F32 = mybir.dt.float32
BF16 = mybir.dt.bfloat16
AF = mybir.ActivationFunctionType
ALU = mybir.AluOpType

KT = N // 128    # 8 k-tiles
QS = N // 128    # 8 q-slices
DT = DIM // 128  # 4 d-tiles
VW = 257         # v-hat width: 256 v + 1 ones (softmax denominator)


def build_program(use_qk_bias=False, use_v_bias=False, use_pb=False):
    nc = bacc.Bacc("TRN2", target_bir_lowering=False, debug=True)

    x_d = nc.declare_dram_parameter("x", [BL, N, DIM], F32, isOutput=False)
    wqk_d = nc.declare_dram_parameter("wqk", [DIM, H * 128], BF16, isOutput=False)
    wv_d = nc.declare_dram_parameter("wv", [DIM, DH], BF16, isOutput=False)
    bqk_d = nc.declare_dram_parameter("bqk", [1, H * 128], BF16, isOutput=False)
    bv_d = nc.declare_dram_parameter("bv", [1, H * 256], BF16, isOutput=False)
    pw_d = nc.declare_dram_parameter("pw", [DH, DIM], BF16, isOutput=False)
    pb1_d = nc.declare_dram_parameter("pb1", [1, DIM], BF16, isOutput=False)
    eb_d = nc.declare_dram_parameter("eb", [H, N, N], BF16, isOutput=False)
    identb_d = nc.declare_dram_parameter("identb", [128, 128], BF16, isOutput=False)
    ones_d = nc.declare_dram_parameter("ones", [1, 512], BF16, isOutput=False)
    y_d = nc.declare_dram_parameter("y", [BL, N, DIM], F32, isOutput=True)

    with tile.TileContext(nc) as tc:
        with (
            tc.tile_pool(name="consts", bufs=1) as cpool,
            tc.tile_pool(name="xnt", bufs=2) as xpool,
            tc.tile_pool(name="lnx", bufs=4) as xtpool,
            tc.tile_pool(name="lnxn", bufs=5) as xnpool,
            tc.tile_pool(name="stats", bufs=16) as spool,
            tc.tile_pool(name="eb", bufs=2) as ebpool,
            tc.tile_pool(name="qk", bufs=4) as qkpool,
            tc.tile_pool(name="vhat", bufs=3) as vpool,
            tc.tile_pool(name="expst", bufs=16) as epool,
            tc.tile_pool(name="anq", bufs=2) as aqpool,
            tc.tile_pool(name="attnT", bufs=14) as atpool,
            tc.tile_pool(name="yout", bufs=2) as ypool,
            tc.tile_pool(name="stp", bufs=2, space="PSUM") as stpp,
            tc.tile_pool(name="pvp", bufs=2, space="PSUM") as pvpp,
            tc.tile_pool(name="miscp", bufs=2, space="PSUM") as mpp,
            tc.tile_pool(name="dram", bufs=1, space="DRAM") as dpool,
        ):
            # ---- constants (x/identb loads first; pw deferred) ----
            identb = cpool.tile([128, 128], BF16)
            nc.sync.dma_start(identb[:], identb_d[:])
            eps_t = cpool.tile([128, 1], F32)
            nc.vector.memset(eps_t[:], 1e-5)
            zero_t = cpool.tile([128, 1], F32)
            nc.vector.memset(zero_t[:], 0.0)
            if use_qk_bias or use_v_bias or use_pb:
                ones_bf = cpool.tile([1, 512], BF16)
                nc.sync.dma_start(ones_bf[:], ones_d[:])
            if use_qk_bias:
                bqk = cpool.tile([1, H * 128], BF16)
                nc.sync.dma_start(bqk[:], bqk_d[:])
            if use_v_bias:
                bv = cpool.tile([1, H * 256], BF16)
                nc.sync.dma_start(bv[:], bv_d[:])
            if use_pb:
                pb1 = cpool.tile([1, DIM], BF16)
                nc.sync.dma_start(pb1[:], pb1_d[:])
            wqk = cpool.tile([128, DT * H * 128], BF16)  # [d-tile][dpart, f]
            wv = cpool.tile([128, DT * DH], BF16)
            pw = cpool.tile([128, 16 * DIM], BF16)
            slab2 = cpool.tile([128, 2 * N], BF16)  # last head's attnT

            def emit_weight_loads():
                for dt in range(DT):
                    for hh in range(2):
                        nc.sync.dma_start(
                            wqk[:, dt * H * 128 + hh * 512:
                                dt * H * 128 + (hh + 1) * 512],
                            wqk_d[dt * 128:(dt + 1) * 128,
                                  hh * 512:(hh + 1) * 512],
                        )
                for dt in range(DT):
                    for hh in range(2):
                        nc.sync.dma_start(
                            wv[:, dt * DH + hh * (DH // 2):
                               dt * DH + (hh + 1) * (DH // 2)],
                            wv_d[dt * 128:(dt + 1) * 128,
                                 hh * (DH // 2):(hh + 1) * (DH // 2)],
                        )

            def emit_pw_loads():
                for dh in range(16):
                    nc.sync.dma_start(
                        pw[:, dh * DIM:(dh + 1) * DIM],
                        pw_d[dh * 128:(dh + 1) * 128, :],
                    )
            # DRAM ring for the normalized attention (two batches deep)
            attn_dram = dpool.tile([2, N, DH], BF16)

            xnt_tiles = {}

            def get_xnt(b):
                if b not in xnt_tiles:
                    xnt_tiles[b] = xpool.tile(
                        [128, DT * N], BF16, tag="xnt", name="xnt"
                    )
                return xnt_tiles[b]

            def emit_x(b, sl):
                """Issue the x-tile DMA for slice sl of batch b."""
                xt = xtpool.tile([128, DIM], F32, tag="x", name="xt")
                nc.sync.dma_start(xt[:], x_d[b, sl * 128:(sl + 1) * 128, :])
                return xt

            def emit_ln_stats(b, sl, xt):
                """LayerNorm compute (no PE) for a prefetched x tile."""
                st6 = spool.tile([128, 6], F32, tag="st6")
                nc.vector.bn_stats(st6[:], xt[:])
                mv = spool.tile([128, 2], F32, tag="mv")
                nc.vector.bn_aggr(mv[:], st6[:])
                sd = spool.tile([128, 1], F32, tag="sd")
                nc.scalar.activation(sd[:], mv[:, 1:2], AF.Sqrt, bias=eps_t[:])
                rs = spool.tile([128, 1], F32, tag="rs")
                nc.vector.reciprocal(rs[:], sd[:])
                nm = spool.tile([128, 1], F32, tag="nm")
                nc.vector.tensor_scalar(
                    nm[:], mv[:, 0:1], rs[:], -1.0, ALU.mult, ALU.mult
                )
                xn = xnpool.tile([128, DIM], BF16, tag="xn", name="xn")
                nc.vector.tensor_scalar(
                    xn[:], xt[:], rs[:], nm[:], ALU.mult, ALU.add
                )
                return xn

            def emit_ln_tp(b, sl, xn):
                """PE transposes of a prepared LN slice into xnT."""
                xnt = get_xnt(b)
                for dt in range(DT):
                    tp = mpp.tile([128, 128], BF16, tag="m", name="lntp")
                    nc.tensor.transpose(
                        tp[:], xn[:, dt * 128:(dt + 1) * 128], identb[:]
                    )
                    nc.vector.tensor_copy(
                        xnt[:, dt * N + sl * 128: dt * N + (sl + 1) * 128],
                        tp[:],
                    )

            def emit_ln(b, sl):
                emit_ln_tp(b, sl, emit_ln_stats(b, sl, emit_x(b, sl)))

            def emit_score_kt(hctx, est, kt):
                """One k-tile of transposed scores + exp + bias-multiply."""
                qt, ktt, vh, ebh = hctx
                sp = stpp.tile([128, N], F32, tag="st")
                ks = ktt[:, kt * 128:(kt + 1) * 128]
                nc.tensor.matmul(
                    sp[:, 0:512], ks, qt[:, 0:512], start=True, stop=True,
                )
                nc.tensor.matmul(
                    sp[:, 512:1024], ks, qt[:, 512:1024],
                    start=True, stop=True, skip_group_check=True,
                )
                et = epool.tile([128, N], BF16, tag="e")
                nc.scalar.activation(et[:], sp[:], AF.Exp, bias=zero_t[:])
                nc.vector.tensor_tensor(
                    et[:], et[:],
                    ebh[kt // 4][:, (kt % 4) * N:(kt % 4 + 1) * N], ALU.mult,
                )
                est.append(et)

            def emit_qp(b, h, qt, ktt, c):
                """qT/kT chunk c for head h (into partitions 0:64 tiles)."""
                xnt = get_xnt(b)
                qp = mpp.tile([128, 512], F32, tag="m", name="qp")
                for dt in range(DT):
                    nc.tensor.matmul(
                        qp[:],
                        wqk[:, dt * H * 128 + h * 128:
                            dt * H * 128 + (h + 1) * 128],
                        xnt[:, dt * N + c * 512: dt * N + (c + 1) * 512],
                        start=(dt == 0),
                        stop=(not use_qk_bias and dt == DT - 1),
                    )
                if use_qk_bias:
                    nc.tensor.matmul(
                        qp[:],
                        bqk[:, h * 128:(h + 1) * 128],
                        ones_bf[:, 0:512],
                        start=False,
                        stop=True,
                    )
                nc.vector.tensor_copy(qt[:, c * 512:(c + 1) * 512], qp[0:64, :])
                nc.vector.tensor_copy(ktt[:, c * 512:(c + 1) * 512],
                                      qp[64:128, :])

            def emit_v(b, h, vh, sl):
                """v-hat slice sl for head h."""
                xnt = get_xnt(b)
                vp = pvpp.tile([128, VW], F32, tag="pv", name="vp")
                for dt in range(DT):
                    nc.tensor.matmul(
                        vp[:, 0:256],
                        xnt[:, dt * N + sl * 128: dt * N + (sl + 1) * 128],
                        wv[:, dt * DH + h * 256: dt * DH + (h + 1) * 256],
                        start=(dt == 0),
                        stop=(not use_v_bias and dt == DT - 1),
                    )
                if use_v_bias:
                    nc.tensor.matmul(
                        vp[:, 0:256],
                        ones_bf[:, 0:128],
                        bv[:, h * 256:(h + 1) * 256],
                        start=False,
                        stop=True,
                        skip_group_check=True,
                    )
                nc.scalar.copy(vh[:, sl * VW: sl * VW + 256], vp[:, 0:256])

            def emit_eb(nh):
                """Exp-bias DMA prefetch for head nh (two half-head tiles)."""
                halves = []
                for hf in range(2):
                    ebh = ebpool.tile([128, 4 * N], BF16, tag="eb", name="ebh")
                    for g in range(2):  # 2 k-tiles per DMA
                        nc.sync.dma_start(
                            ebh[:, g * 2 * N:(g + 1) * 2 * N]
                            .rearrange("p (kt q) -> p kt q", q=N),
                            eb_d[nh, hf * 512 + g * 256:
                                 hf * 512 + (g + 1) * 256, :]
                            .rearrange("(kt p) q -> p kt q", p=128),
                        )
                    halves.append(ebh)
                return halves

            def emit_qkv_tiles(nh):
                qt2 = qkpool.tile([64, N], BF16, tag="qt")
                ktt2 = qkpool.tile([64, N], BF16, tag="kt")
                vh2 = vpool.tile([128, KT * VW], BF16, tag="vh")
                nc.vector.memset(
                    vh2[:].rearrange("p (s w) -> p s w", w=VW)[:, :, 256:257],
                    1.0,
                )
                return qt2, ktt2, vh2

            def emit_qkv_mm(nb, nh, nctx):
                qt2, ktt2, vh2, _ = nctx
                emit_qp(nb, nh, qt2, ktt2, 0)
                emit_qp(nb, nh, qt2, ktt2, 1)
                for sl in range(QS):
                    emit_v(nb, nh, vh2, sl)

            def emit_v_mm(nb, nh, nctx):
                vh2 = nctx[2]
                for sl in range(QS):
                    emit_v(nb, nh, vh2, sl)

            def emit_pv_sl(hctx, est, anq, sl):
                """One q-slice of PV with fused denominator + normalize."""
                qt, ktt, vh, ebh = hctx
                pv = pvpp.tile([128, VW], F32, tag="pv", name="pv")
                for kt in range(KT):
                    nc.tensor.matmul(
                        pv[:],
                        est[kt][:, sl * 128:(sl + 1) * 128],
                        vh[:, kt * VW:(kt + 1) * VW],
                        start=(kt == 0),
                        stop=(kt == KT - 1),
                    )
                rc = spool.tile([128, 1], F32, tag="rc")
                nc.vector.reciprocal(rc[:], pv[:, 256:257])
                nc.vector.tensor_scalar(
                    anq[:, sl * 256:(sl + 1) * 256],
                    pv[:, 0:256], rc[:], None, ALU.mult,
                )

            def emit_head_main(sctx, est_next, hctx, est, anq):
                """Interleave scores/exp of head i+1 with PV of head i:
                the PV matmuls keep the PE busy while the ACT exp chain
                drains the score PSUM tiles."""
                if sctx is not None:
                    emit_score_kt(sctx, est_next, 0)
                    emit_score_kt(sctx, est_next, 1)
                    for kt in range(2, KT):
                        emit_pv_sl(hctx, est, anq, kt - 2)
                        emit_score_kt(sctx, est_next, kt)
                    emit_pv_sl(hctx, est, anq, 6)
                    emit_pv_sl(hctx, est, anq, 7)
                else:
                    for sl in range(QS):
                        emit_pv_sl(hctx, est, anq, sl)

            def emit_attn_write(b, h, anq):
                rb = b % 2
                nc.sync.dma_start(
                    attn_dram[rb, :, h * 256:(h + 1) * 256]
                    .rearrange("(s p) d -> p s d", p=128),
                    anq[:].rearrange("p (s d) -> p s d", d=256),
                )

            def emit_attn_reads(b, h, attns):
                rb = b % 2
                for dc in range(2):
                    at = atpool.tile([128, N], BF16, tag="at")
                    nc.sync.dma_start_transpose(
                        at[:],
                        attn_dram[rb, :, (h * 2 + dc) * 128:
                                  (h * 2 + dc + 1) * 128],
                    )
                    attns.append((at, 0))

            def emit_attn_tb_last(anq, attns):
                """PE transpose-back for the last head (avoids the DRAM
                round-trip latency right before proj)."""
                for sl in range(QS):
                    for dc in range(2):
                        tp = mpp.tile([128, 128], BF16, tag="m", name="tb")
                        nc.tensor.transpose(
                            tp[:],
                            anq[:, sl * 256 + dc * 128:
                                sl * 256 + (dc + 1) * 128],
                            identb[:],
                        )
                        nc.vector.tensor_copy(
                            slab2[:, dc * N + sl * 128:
                                  dc * N + (sl + 1) * 128],
                            tp[:],
                        )
                attns.append((slab2, 0))
                attns.append((slab2, N))

            def emit_proj(b, attns):
                for sl in range(QS):
                    pp = mpp.tile([128, 512], F32, tag="m", name="pp")
                    for dh in range(16):
                        t, base = attns[dh]
                        nc.tensor.matmul(
                            pp[:],
                            t[:, base + sl * 128: base + (sl + 1) * 128],
                            pw[:, dh * DIM:(dh + 1) * DIM],
                            start=(dh == 0),
                            stop=(not use_pb and dh == 15),
                        )
                    if use_pb:
                        nc.tensor.matmul(
                            pp[:], ones_bf[:, 0:128], pb1[:], start=False,
                            stop=True, skip_group_check=True,
                        )
                    yt = ypool.tile([128, DIM], F32, tag="y")
                    nc.scalar.copy(yt[:], pp[:])
                    nc.sync.dma_start(y_d[b, sl * 128:(sl + 1) * 128, :], yt[:])

            # ---- main pipeline ----
            # Global head index i = b*H + h.  Software pipeline depth 2:
            # scores/exp for head i+1 (interleaved with qkv of head i+2)
            # are emitted before PV(i), so the ACT exp chain of i+1
            # executes during PV(i)/v(i+2) and never gates PV(i+1).
            NH = BL * H

            def bh(i):
                return i // H, i % H

            for sl in range(QS):
                emit_ln(0, sl)
            emit_weight_loads()
            hctxs = {0: emit_qkv_tiles(0) + (emit_eb(0),)}
            emit_qkv_mm(0, 0, hctxs[0])
            ests = {0: []}
            for kt in range(KT):
                emit_score_kt(hctxs[0], ests[0], kt)
            hctxs[1] = emit_qkv_tiles(1) + (emit_eb(1),)
            emit_qkv_mm(*bh(1), hctxs[1])
            emit_pw_loads()
            # Next-batch LayerNorm staging: x-DMA two steps before the PE
            # transposes, stats in between, so the in-order DVE queue never
            # blocks on an x load.
            X_SCHED = {0: [0, 1], 1: [2, 3], 2: [4, 5], 3: [6, 7]}
            ST_SCHED = {1: [0, 1], 2: [2, 3], 3: [4, 5], 4: [6, 7]}
            attns = []
            pending_tp = []
            xts = {}
            for i in range(NH):
                b, h = bh(i)
                # PE transposes of last step's prepared LN slices
                for sl, xn in pending_tp:
                    emit_ln_tp(b + 1, sl, xn)
                pending_tp = []
                if b + 1 < BL:
                    for sl in X_SCHED.get(h, []):
                        xts[sl] = emit_x(b + 1, sl)
                if h > 0:
                    emit_attn_reads(b, h - 1, attns)
                # exp-bias prefetch for head i+2
                if i + 2 < NH:
                    hctxs[i + 2] = emit_qkv_tiles(bh(i + 2)[1]) + (
                        emit_eb(bh(i + 2)[1]),
                    )
                # interleaved scores(i+1) + PV(i)
                anq = aqpool.tile([128, QS * 256], BF16, tag="anq")
                sctx = hctxs[i + 1] if i + 1 < NH else None
                est_next = []
                emit_head_main(sctx, est_next, hctxs.pop(i), ests.pop(i), anq)
                if i + 1 < NH:
                    ests[i + 1] = est_next
                if h == H - 1:
                    emit_attn_tb_last(anq, attns)
                else:
                    emit_attn_write(b, h, anq)
                # qkv matmuls for head i+2 (tail of the step)
                if i + 2 < NH:
                    emit_qkv_mm(*bh(i + 2), hctxs[i + 2])
                if h == H - 1:
                    emit_proj(b, attns)
                    attns = []
                    xnt_tiles.pop(b, None)
                # LN stats last (slack: needed a step later)
                if b + 1 < BL:
                    for sl in ST_SCHED.get(h, []):
                        pending_tp.append(
                            (sl, emit_ln_stats(b + 1, sl, xts.pop(sl)))
                        )

    nc.compile()
    return nc


_CACHE = {}


def _prep_host(gamma, beta, qkv_w, qkv_b, proj_w, proj_b, biases, bias_idxs):
    import ml_dtypes

    qkv_w = np.asarray(qkv_w, np.float32)
    qkv_b = np.asarray(qkv_b, np.float32)
    gamma = np.asarray(gamma, np.float32)
    beta = np.asarray(beta, np.float32)
    w = qkv_w * gamma[:, None]          # fold LN gamma
    bfold = qkv_b + beta @ qkv_w        # fold LN beta
    w3 = w.reshape(DIM, H, 384)
    b3 = bfold.reshape(H, 384)
    # q/k columns, q scaled by SCALE
    wqk = np.concatenate([w3[:, :, :64] * SCALE, w3[:, :, 64:128]], axis=2)
    wqk = wqk.reshape(DIM, H * 128)
    bqk = np.concatenate([b3[:, :64] * SCALE, b3[:, 64:128]], axis=1)
    bqk = bqk.reshape(1, H * 128)
    wv = w3[:, :, 128:].reshape(DIM, DH)
    bv = b3[:, 128:].reshape(1, H * 256)
    bias_full = np.asarray(biases, np.float32)[:, np.asarray(bias_idxs)]
    # device reads bias tiles as [k, q]; transpose (a no-op for the
    # symmetric relative-position bias, but correct in general)
    eb = np.exp(bias_full.transpose(0, 2, 1))
    return {
        "wqk": wqk.astype(ml_dtypes.bfloat16),
        "wv": wv.astype(ml_dtypes.bfloat16),
        "bqk": bqk.astype(ml_dtypes.bfloat16),
        "bv": bv.astype(ml_dtypes.bfloat16),
        "pw": np.ascontiguousarray(np.asarray(proj_w, np.float32)).astype(ml_dtypes.bfloat16),
        "pb1": np.asarray(proj_b, np.float32).reshape(1, DIM).astype(ml_dtypes.bfloat16),
        "eb": np.ascontiguousarray(eb).astype(ml_dtypes.bfloat16),
        "identb": np.eye(128, dtype=np.float32).astype(ml_dtypes.bfloat16),
        "ones": np.ones((1, 512), ml_dtypes.bfloat16),
    }


def kernel(x, gamma, beta, qkv_w, qkv_b, proj_w, proj_b, biases, bias_idxs,
           _trace=False, _tmpdir=None):
    x = np.asarray(x, np.float32)
    shared = _prep_host(gamma, beta, qkv_w, qkv_b, proj_w, proj_b, biases,
                        bias_idxs)
    flags = (
        bool(np.any(np.asarray(shared["bqk"], np.float32))),
        bool(np.any(np.asarray(shared["bv"], np.float32))),
        bool(np.any(np.asarray(shared["pb1"], np.float32))),
    )
    if _CACHE.get("flags") != flags:
        _CACHE["nc"] = build_program(*flags)
        _CACHE["flags"] = flags
    nc = _CACHE["nc"]
    in_maps = []
    for c in range(NCORES):
        m = dict(shared)
        m["x"] = np.ascontiguousarray(x[c * BL:(c + 1) * BL])
        in_maps.append(m)
    res = run_bass_kernel_spmd(
        nc, in_maps, list(range(NCORES)), trace=_trace, tmpdir=_tmpdir,
    )
    _CACHE["last"] = res
    out = np.concatenate([res.results[c]["y"] for c in range(NCORES)], axis=0)
    return out.astype(np.float32)
